# revision 37
# baseline (speedup 1.0000x reference)
"""GCNConv (out = segsum((X@W)[col], row)) on 8 TRN2 NeuronCores — v3.

v2 aggregated in D_in space: it streamed host-gathered neighbor rows at
128 feats/edge (bf16, ~29MB/core) and was DMA-bound at ~104us
(~330GB/s/core HBM), with 61us of DVE mask generation and 40us of ACT
copy overhead hidden under the stream.

v3 transforms FIRST so the gathered stream carries D_out=64 feats/edge
— half the bytes — and restructures so no masks are needed at all:

  Launch A (~10us): X' = X @ W, node-sharded (core k owns rows
    [6250k, 6250(k+1))), W stationary, X'^T written straight from PSUM.
  Host (index ops only): destinations sorted by degree and dealt
    round-robin into 128-dest blocks, so same-rank blocks across cores
    have near-equal tile counts (shared SPMD program, ~2.5% padding);
    X'[col] is gathered per edge into a slotted lane-major stream where
    lane l of EVERY tile belongs to dest l of the block.
  Launch B (~45us): stream Xg' [128, NT*64] bf16; the segment sum for a
    block is plain PSUM accumulation of its tiles under an IDENTITY
    stationary (one [128,128] lhsT reused by all 802 matmuls): no
    per-tile DVE masks, no rr stream, one DVE copy + one out-DMA per
    7-block chunk.

Precision: bf16 stream/weights, f32 PSUM accumulation, bf16 out (host
casts to f32): rel err ~2.5e-3 vs the 2e-2 gate.
"""

import numpy as np
import ml_dtypes

import concourse.mybir as mybir
import concourse.tile as tile
from concourse import bacc
from concourse.bass_utils import run_bass_kernel_spmd

# ---- problem constants (must match the harness inputs) ----
N_NODES = 50000
N_EDGES = 800000
D_IN = 128
D_OUT = 64
N_CORES = 8

NPC = N_NODES // N_CORES                    # 6250: nodes/core in launch A
BLK = 128                                   # dests per block in launch B
NBLK = (N_NODES + BLK - 1) // BLK           # 391 dest blocks
SLOTS = (NBLK + N_CORES - 1) // N_CORES     # 49 block slots per core
CHUNK_SLOTS = 7                             # blocks per chunk (psum 1792B)
N_CHUNKS = SLOTS // CHUNK_SLOTS             # 7
# slot processing order: the smallest slot first (fast PE start), then
# descending sizes; out[:, p, :] holds slot P_ORDER[p] (host relabels)
P_ORDER = [SLOTS - 1] + list(range(SLOTS - 1))
A_N = 512                                   # launch-A moving width
A_NCH = (NPC + A_N - 1) // A_N              # 13

ST_DT = mybir.dt.bfloat16
NP_ST = ml_dtypes.bfloat16

# test.py can flip this to get a profiled run; results land in LAST_RESULTS.
TRACE = False
LAST_RESULTS = None                         # [res_a, res_b]

# feature split of the Xg' stream: FH leading features in bf16, FL=64-FH
# trailing features in fp8e4m3 (exact 0/1 identity, quantization error only
# on the fp8 block: rel err ~2.65e-2 * sqrt(FL/64) end-to-end).  (64, 0)
# disables fp8.  Requires non-self-loading matmuls (the per-chunk identity
# loads): stationary dtype alternates per pass, so per-matmul reloads
# would make the PE LDW-bound.
FH, FL = 64, 0


def build_program_a():
    """X' = X @ W for this core's 6250-node slice, node-major output.

    Node-major (out partition = node) keeps all 128 DVE/ACT lanes busy in
    the psum->sbuf copies (2x the elems/cycle of the 64-partition
    W-stationary orientation) and the host gather reads rows directly.
    """
    nc = bacc.Bacc("TRN2", target_bir_lowering=False, debug=False,
                   num_devices=N_CORES)
    xt = nc.dram_tensor("xt", [D_IN, NPC], ST_DT, kind="ExternalInput").ap()
    w = nc.dram_tensor("w", [D_IN, D_OUT], ST_DT, kind="ExternalInput").ap()
    NCH = (NPC + BLK - 1) // BLK               # 49 chunks of 128 nodes
    GCH = 8                                    # chunks per group (1 bank)
    # out stays in sbuf layout [p, chunk, f] (node n = chunk*128 + p, host
    # reshapes): per-partition contiguous runs ~1KB, no <512B DMA penalty
    xp = nc.dram_tensor("xp", [BLK, NCH, D_OUT], ST_DT,
                        kind="ExternalOutput").ap()
    with tile.TileContext(nc) as tc:
        with (
            tc.tile_pool(name="const", bufs=1) as cpool,
            tc.tile_pool(name="xt", bufs=1) as xpool,
            tc.tile_pool(name="ps", bufs=5, space="PSUM") as psum,
            tc.tile_pool(name="wrm", bufs=1, space="PSUM") as wpool,
            tc.tile_pool(name="xo", bufs=4) as opool,
        ):
            w_sb = cpool.tile([D_IN, D_OUT], ST_DT)
            nc.sync.dma_start(w_sb[:], w[:])
            xt_sb = xpool.tile([D_IN, NPC], ST_DT)
            # alternate input DMAs across the two HWDGE queues: the ~0.6us
            # per-DMA sequencer issue time was serializing the input stream
            for i, n0 in enumerate(range(0, NPC, GCH * BLK)):
                ng = min(GCH * BLK, NPC - n0)
                eng = nc.sync if i % 2 == 0 else nc.scalar
                eng.dma_start(xt_sb[:, n0:n0 + ng], xt[:, n0:n0 + ng])
            # PE p-state warm-up: ~3us of back-to-back dummy matmuls on w
            # (which lands well before the first input group) ramp the PE to
            # full clock before the real transform begins
            wp = wpool.tile([D_OUT, D_OUT], mybir.dt.float32, tag="warm")
            for _ in range(40):
                nc.tensor.matmul(out=wp[:], lhsT=w_sb[:], rhs=w_sb[:],
                                 start=True, stop=True)
            for g in range(0, NCH, GCH):
                nch = min(GCH, NCH - g)
                ps = psum.tile([BLK, GCH, D_OUT], mybir.dt.float32, tag="ps")
                for c in range(nch):
                    n0 = (g + c) * BLK
                    nn = min(BLK, NPC - n0)
                    nc.tensor.matmul(out=ps[:nn, c, :], lhsT=xt_sb[:, n0:n0 + nn],
                                     rhs=w_sb[:], start=True, stop=True)
                xo = opool.tile([BLK, GCH, D_OUT], ST_DT, tag="xo")
                np_ = min(BLK, NPC - (g + nch - 1) * BLK)  # last-chunk rows
                np_ = BLK if nch > 1 else np_
                if (g // GCH) % 2 == 0:
                    nc.vector.tensor_copy(out=xo[:np_, :nch, :],
                                          in_=ps[:np_, :nch, :])
                else:
                    nc.scalar.copy(xo[:np_, :nch, :], ps[:np_, :nch, :])
                eng = nc.sync if (g // GCH) % 2 == 0 else nc.scalar
                eng.dma_start(xp[:np_, g:g + nch, :], xo[:np_, :nch, :])
    nc.compile()
    return nc


FP8_DT = mybir.dt.float8e4
NP_FP8 = ml_dtypes.float8_e4m3fn


def build_program_b(T_list):
    """Segment-sum of the slotted Xg' stream: identity-stationary matmuls.

    T_list[s] = tiles for block slot s (uniform across cores; processing
    order).  Per chunk: a bf16 pass (FH leading feats, per-slot HWDGE
    DMAs on SP) and an fp8 pass (FL trailing feats, per-chunk DMAs on the
    DVE queue), each under ONE explicit identity ldweights; the matmuls
    are non-self-loading.
    """
    T_list = [int(t) for t in T_list]
    off = np.concatenate([[0], np.cumsum(T_list)]).astype(int)
    nc = bacc.Bacc("TRN2", target_bir_lowering=False, debug=False,
                   num_devices=N_CORES)
    NT = int(off[-1])
    xgh = nc.dram_tensor("xgh", [BLK, NT * FH], ST_DT,
                         kind="ExternalInput").ap()
    identb = nc.dram_tensor("identb", [BLK, BLK], ST_DT,
                            kind="ExternalInput").ap()
    if FL:
        xgl = nc.dram_tensor("xgl", [BLK, NT * FL], FP8_DT,
                             kind="ExternalInput").ap()
        identf = nc.dram_tensor("identf", [BLK, BLK], FP8_DT,
                                kind="ExternalInput").ap()
    # out[lane, p, f']; host maps (lane, p) -> node via P_ORDER/degree sort
    out = nc.dram_tensor("out", [BLK, SLOTS, D_OUT], ST_DT,
                         kind="ExternalOutput").ap()

    with tile.TileContext(nc) as tc:
        with (
            tc.tile_pool(name="const", bufs=1) as cpool,
            tc.tile_pool(name="xgh", bufs=14) as xhpool,
            tc.tile_pool(name="xgl", bufs=3) as xlpool,
            tc.tile_pool(name="agg", bufs=6, space="PSUM") as apsum,
            tc.tile_pool(name="ob", bufs=4) as opool,
        ):
            def slot_dma_h(s0, ns):
                ts = int(off[s0])
                nts = int(off[s0 + ns]) - ts
                t_ = xhpool.tile([BLK, nts * FH], ST_DT, tag="xh")
                # alternate HWDGE queues: overlaps the ~0.6us issue and
                # ~0.6us HWDGE fixed cost across two contexts
                eng = nc.sync if s0 % 4 < 2 else nc.scalar
                eng.dma_start(t_[:], xgh[:, ts * FH:(ts + nts) * FH])
                return t_

            def chunk_dma_l(s0):
                ts = int(off[s0])
                nts = int(off[s0 + CHUNK_SLOTS]) - ts
                t_ = xlpool.tile([BLK, nts * FL], FP8_DT, tag="xl")
                nc.scalar.dma_start(t_[:], xgl[:, ts * FL:(ts + nts) * FL])
                return t_

            # stream DMA units: slot 0 alone (small, fast PE start), then
            # slot pairs — 25 DMAs instead of 49 halves issue/HWDGE costs
            units = [(0, 1)] + [(s, min(2, SLOTS - s))
                                for s in range(1, SLOTS, 2)]
            unit_of = {}
            unit_base = {}
            for ui, (s0, ns) in enumerate(units):
                for b in range(ns):
                    unit_of[s0 + b] = ui
                    unit_base[s0 + b] = s0
            unit_tiles = {0: slot_dma_h(*units[0])}
            chunk_l = {0: chunk_dma_l(0)} if FL else {}
            identb_sb = cpool.tile([BLK, BLK], ST_DT)
            nc.sync.dma_start(identb_sb[:], identb[:])
            if FL:
                identf_sb = cpool.tile([BLK, BLK], FP8_DT)
                nc.sync.dma_start(identf_sb[:], identf[:])

            def noload(mm):
                mm.ins.ldweights = False
                return mm

            def emit_out(s0, ps):
                ob = opool.tile([BLK, CHUNK_SLOTS, D_OUT], ST_DT, tag="ob")
                nc.vector.tensor_copy(out=ob[:], in_=ps[:])
                nc.scalar.dma_start(out[:, s0:s0 + CHUNK_SLOTS, :], ob[:])

            prev = None
            for ci in range(N_CHUNKS):
                s0 = ci * CHUNK_SLOTS
                for b in range(CHUNK_SLOTS):
                    ui = unit_of[s0 + b]
                    if ui not in unit_tiles:
                        unit_tiles[ui] = slot_dma_h(*units[ui])
                if FL and ci + 1 < N_CHUNKS and (s0 + CHUNK_SLOTS) not in chunk_l:
                    chunk_l[s0 + CHUNK_SLOTS] = chunk_dma_l(s0 + CHUNK_SLOTS)
                ps = apsum.tile([BLK, CHUNK_SLOTS, D_OUT], mybir.dt.float32,
                                tag="ps")
                if prev is not None:
                    # chunk ci-1's copy/store: deps a chunk old, stall-free
                    emit_out(*prev)
                # bf16 pass: one stationary load for the whole chunk
                nc.tensor.ldweights(identb_sb[:])
                for b in range(CHUNK_SLOTS):
                    s = s0 + b
                    xh_t = unit_tiles[unit_of[s]]
                    tb = int(off[s]) - int(off[unit_base[s]])
                    for t in range(T_list[s]):
                        ti = tb + t
                        noload(nc.tensor.matmul(
                            out=ps[:, b, 0:FH], lhsT=identb_sb[:],
                            rhs=xh_t[:, ti * FH:(ti + 1) * FH],
                            start=(t == 0), stop=(t == T_list[s] - 1)))
                if FL:
                    # fp8 pass
                    xl_t = chunk_l.pop(s0)
                    nc.tensor.ldweights(identf_sb[:])
                    for b in range(CHUNK_SLOTS):
                        s = s0 + b
                        toff = int(off[s]) - int(off[s0])
                        for t in range(T_list[s]):
                            ti = toff + t
                            noload(nc.tensor.matmul(
                                out=ps[:, b, FH:D_OUT], lhsT=identf_sb[:],
                                rhs=xl_t[:, ti * FL:(ti + 1) * FL],
                                start=(t == 0), stop=(t == T_list[s] - 1)))
                prev = (s0, ps)
            emit_out(*prev)
    nc.compile()
    return nc


def prepare(row_index, column_index):
    """Host-side index-only planning: degree sort, block deal, slotting."""
    row = np.ascontiguousarray(row_index).astype(np.int64)
    col = np.ascontiguousarray(column_index).astype(np.int64)
    deg = np.bincount(row, minlength=N_NODES)
    order = np.argsort(-deg, kind="stable")          # rank -> node
    rank = np.empty(N_NODES, np.int64)
    rank[order] = np.arange(N_NODES)
    ds = deg[order]
    # block j's max degree is its first member (descending sort)
    T_blk = np.maximum(ds[::BLK], 1)                 # [NBLK]
    # slot s on every core holds one of blocks 8s..8s+7; block 8s is the
    # largest, so T_blk[8s] covers all cores.  The xg layout and T_list
    # follow P_ORDER (processing order: smallest slot first).
    T_slot = T_blk[::N_CORES].astype(np.int64)       # [SLOTS], by slot id
    pinv = np.empty(SLOTS, np.int64)
    pinv[np.asarray(P_ORDER)] = np.arange(SLOTS)     # slot -> processed pos
    T_list = T_slot[np.asarray(P_ORDER)]             # by processed pos
    off = np.concatenate([[0], np.cumsum(T_list)]).astype(np.int64)
    NT = int(off[-1])

    r = rank[row]
    j = r // BLK                                     # dest block
    lane = r % BLK
    core = j % N_CORES
    pos = pinv[j // N_CORES]                         # processed position
    starts = np.concatenate([[0], np.cumsum(deg)]).astype(np.int64)
    occ = np.arange(N_EDGES, dtype=np.int64) - starts[row]
    tilei = off[pos] + occ                           # occ < T_list[pos]
    gidx = np.full((N_CORES, BLK, NT), -1, np.int64)
    gidx[core, lane, tilei] = col
    return {"order": order, "T_list": T_list, "gidx": gidx, "NT": NT}


def inputs_a(X, weights):
    X_bf = np.ascontiguousarray(X).astype(NP_ST)
    w_bf = np.ascontiguousarray(weights).astype(NP_ST)
    return [{"xt": np.ascontiguousarray(X_bf[k * NPC:(k + 1) * NPC].T),
             "w": w_bf} for k in range(N_CORES)]


def inputs_b(xp_any, P):
    """Gather X'[col] into the slotted lane-major streams per core (bf16
    leading FH features; fp8 trailing FL features)."""
    xp_bf = np.ascontiguousarray(xp_any).astype(NP_ST)
    xp_hi = np.ascontiguousarray(xp_bf[:, :FH])
    if FL:
        xp_lo = np.ascontiguousarray(xp_bf[:, FH:]).astype(NP_FP8)
    identb = np.eye(BLK, dtype=np.float32).astype(NP_ST)
    identf = np.eye(BLK, dtype=np.float32).astype(NP_FP8)
    NT = P["NT"]
    maps = []
    for k in range(N_CORES):
        g = P["gidx"][k].ravel()                     # [BLK*NT]
        gc = np.maximum(g, 0)
        bad = g < 0
        hi = xp_hi[gc]                               # [BLK*NT, FH]
        hi[bad] = 0
        m = {"xgh": np.ascontiguousarray(hi.reshape(BLK, NT * FH)),
             "identb": identb}
        if FL:
            lo = xp_lo[gc]
            lo[bad] = 0
            m["xgl"] = np.ascontiguousarray(lo.reshape(BLK, NT * FL))
            m["identf"] = identf
        maps.append(m)
    return maps


def unshard(P, outs):
    """outs[k]: device out [BLK, SLOTS, D_OUT] -> full [N_NODES, D_OUT]."""
    order = P["order"]
    res = np.zeros((N_NODES, D_OUT), np.float32)
    lanes = np.arange(BLK)[:, None]
    porder = np.asarray(P_ORDER)
    for k in range(N_CORES):
        ob = np.asarray(outs[k], dtype=np.float32)
        jj = porder[None, :] * N_CORES + k               # block of position p
        rk = jj * BLK + lanes                            # [BLK, SLOTS] ranks
        valid = rk < N_NODES
        res[order[rk[valid]]] = ob[valid]
    return res


def kernel(X, weights, row_index, column_index):
    global LAST_RESULTS
    P = prepare(row_index, column_index)
    nc_a = build_program_a()
    res_a = run_bass_kernel_spmd(nc_a, inputs_a(X, weights),
                                 list(range(N_CORES)), trace=TRACE)
    # device xp is [p, chunk, f], node n = chunk*128 + p
    xp = np.concatenate(
        [np.asarray(res_a.results[k]["xp"]).transpose(1, 0, 2)
         .reshape(-1, D_OUT)[:NPC] for k in range(N_CORES)],
        axis=0)                                          # [N_NODES, D_OUT]
    in_b = inputs_b(xp, P)
    nc_b = build_program_b(P["T_list"])
    res_b = run_bass_kernel_spmd(nc_b, in_b,
                                 list(range(N_CORES)), trace=TRACE)
    LAST_RESULTS = [res_a, res_b]
    return unshard(P, [res_b.results[k]["out"] for k in range(N_CORES)])


# revision 38
# speedup vs baseline: 1.0327x; 1.0327x over previous
"""GCNConv (out = segsum((X@W)[col], row)) on 8 TRN2 NeuronCores — v3.

v2 aggregated in D_in space: it streamed host-gathered neighbor rows at
128 feats/edge (bf16, ~29MB/core) and was DMA-bound at ~104us
(~330GB/s/core HBM), with 61us of DVE mask generation and 40us of ACT
copy overhead hidden under the stream.

v3 transforms FIRST so the gathered stream carries D_out=64 feats/edge
— half the bytes — and restructures so no masks are needed at all:

  Launch A (~10us): X' = X @ W, node-sharded (core k owns rows
    [6250k, 6250(k+1))), W stationary, X'^T written straight from PSUM.
  Host (index ops only): destinations sorted by degree and dealt
    round-robin into 128-dest blocks, so same-rank blocks across cores
    have near-equal tile counts (shared SPMD program, ~2.5% padding);
    X'[col] is gathered per edge into a slotted lane-major stream where
    lane l of EVERY tile belongs to dest l of the block.
  Launch B (~45us): stream Xg' [128, NT*64] bf16; the segment sum for a
    block is plain PSUM accumulation of its tiles under an IDENTITY
    stationary (one [128,128] lhsT reused by all 802 matmuls): no
    per-tile DVE masks, no rr stream, one DVE copy + one out-DMA per
    7-block chunk.

Precision: bf16 stream/weights, f32 PSUM accumulation, bf16 out (host
casts to f32): rel err ~2.5e-3 vs the 2e-2 gate.
"""

import numpy as np
import ml_dtypes

import concourse.mybir as mybir
import concourse.tile as tile
from concourse import bacc
from concourse.bass_utils import run_bass_kernel_spmd

# ---- problem constants (must match the harness inputs) ----
N_NODES = 50000
N_EDGES = 800000
D_IN = 128
D_OUT = 64
N_CORES = 8

NPC = N_NODES // N_CORES                    # 6250: nodes/core in launch A
BLK = 128                                   # dests per block in launch B
NBLK = (N_NODES + BLK - 1) // BLK           # 391 dest blocks
SLOTS = (NBLK + N_CORES - 1) // N_CORES     # 49 block slots per core
CHUNK_SLOTS = 7                             # blocks per chunk (psum 1792B)
N_CHUNKS = SLOTS // CHUNK_SLOTS             # 7
# slot processing order: the smallest slot first (fast PE start), then
# descending sizes; out[:, p, :] holds slot P_ORDER[p] (host relabels)
P_ORDER = [SLOTS - 1] + list(range(SLOTS - 1))
A_N = 512                                   # launch-A moving width
A_NCH = (NPC + A_N - 1) // A_N              # 13

ST_DT = mybir.dt.bfloat16
NP_ST = ml_dtypes.bfloat16

# test.py can flip this to get a profiled run; results land in LAST_RESULTS.
TRACE = False
LAST_RESULTS = None                         # [res_a, res_b]

# feature split of the Xg' stream: FH leading features in bf16, FL=64-FH
# trailing features in fp8e4m3 (exact 0/1 identity, quantization error only
# on the fp8 block: rel err ~2.65e-2 * sqrt(FL/64) end-to-end).  (64, 0)
# disables fp8.  Requires non-self-loading matmuls (the per-chunk identity
# loads): stationary dtype alternates per pass, so per-matmul reloads
# would make the PE LDW-bound.
FH, FL = 64, 0


def build_program_a():
    """X' = X @ W for this core's 6250-node slice, node-major output.

    Node-major (out partition = node) keeps all 128 DVE/ACT lanes busy in
    the psum->sbuf copies (2x the elems/cycle of the 64-partition
    W-stationary orientation) and the host gather reads rows directly.
    """
    nc = bacc.Bacc("TRN2", target_bir_lowering=False, debug=False,
                   num_devices=N_CORES)
    xt = nc.dram_tensor("xt", [D_IN, NPC], ST_DT, kind="ExternalInput").ap()
    w = nc.dram_tensor("w", [D_IN, D_OUT], ST_DT, kind="ExternalInput").ap()
    NCH = (NPC + BLK - 1) // BLK               # 49 chunks of 128 nodes
    GCH = 8                                    # chunks per group (1 bank)
    # out stays in sbuf layout [p, chunk, f] (node n = chunk*128 + p, host
    # reshapes): per-partition contiguous runs ~1KB, no <512B DMA penalty
    xp = nc.dram_tensor("xp", [BLK, NCH, D_OUT], ST_DT,
                        kind="ExternalOutput").ap()
    with tile.TileContext(nc) as tc:
        with (
            tc.tile_pool(name="const", bufs=1) as cpool,
            tc.tile_pool(name="xt", bufs=1) as xpool,
            tc.tile_pool(name="ps", bufs=6, space="PSUM") as psum,
            tc.tile_pool(name="xo", bufs=4) as opool,
        ):
            w_sb = cpool.tile([D_IN, D_OUT], ST_DT)
            nc.sync.dma_start(w_sb[:], w[:])
            xt_sb = xpool.tile([D_IN, NPC], ST_DT)
            # alternate input DMAs across the two HWDGE queues: the ~0.6us
            # per-DMA sequencer issue time was serializing the input stream
            for i, n0 in enumerate(range(0, NPC, GCH * BLK)):
                ng = min(GCH * BLK, NPC - n0)
                eng = nc.sync if i % 2 == 0 else nc.scalar
                eng.dma_start(xt_sb[:, n0:n0 + ng], xt[:, n0:n0 + ng])
            for g in range(0, NCH, GCH):
                nch = min(GCH, NCH - g)
                ps = psum.tile([BLK, GCH, D_OUT], mybir.dt.float32, tag="ps")
                for c in range(nch):
                    n0 = (g + c) * BLK
                    nn = min(BLK, NPC - n0)
                    nc.tensor.matmul(out=ps[:nn, c, :], lhsT=xt_sb[:, n0:n0 + nn],
                                     rhs=w_sb[:], start=True, stop=True)
                xo = opool.tile([BLK, GCH, D_OUT], ST_DT, tag="xo")
                np_ = min(BLK, NPC - (g + nch - 1) * BLK)  # last-chunk rows
                np_ = BLK if nch > 1 else np_
                if (g // GCH) % 2 == 0:
                    nc.vector.tensor_copy(out=xo[:np_, :nch, :],
                                          in_=ps[:np_, :nch, :])
                else:
                    nc.scalar.copy(xo[:np_, :nch, :], ps[:np_, :nch, :])
                eng = nc.sync if (g // GCH) % 2 == 0 else nc.scalar
                eng.dma_start(xp[:np_, g:g + nch, :], xo[:np_, :nch, :])
    nc.compile()
    return nc


FP8_DT = mybir.dt.float8e4
NP_FP8 = ml_dtypes.float8_e4m3fn


def build_program_b(T_list):
    """Segment-sum of the slotted Xg' stream: identity-stationary matmuls.

    T_list[s] = tiles for block slot s (uniform across cores; processing
    order).  Per chunk: a bf16 pass (FH leading feats, per-slot HWDGE
    DMAs on SP) and an fp8 pass (FL trailing feats, per-chunk DMAs on the
    DVE queue), each under ONE explicit identity ldweights; the matmuls
    are non-self-loading.
    """
    T_list = [int(t) for t in T_list]
    off = np.concatenate([[0], np.cumsum(T_list)]).astype(int)
    nc = bacc.Bacc("TRN2", target_bir_lowering=False, debug=False,
                   num_devices=N_CORES)
    NT = int(off[-1])
    xgh = nc.dram_tensor("xgh", [BLK, NT * FH], ST_DT,
                         kind="ExternalInput").ap()
    identb = nc.dram_tensor("identb", [BLK, BLK], ST_DT,
                            kind="ExternalInput").ap()
    if FL:
        xgl = nc.dram_tensor("xgl", [BLK, NT * FL], FP8_DT,
                             kind="ExternalInput").ap()
        identf = nc.dram_tensor("identf", [BLK, BLK], FP8_DT,
                                kind="ExternalInput").ap()
    # out[lane, p, f']; host maps (lane, p) -> node via P_ORDER/degree sort
    out = nc.dram_tensor("out", [BLK, SLOTS, D_OUT], ST_DT,
                         kind="ExternalOutput").ap()

    with tile.TileContext(nc) as tc:
        with (
            tc.tile_pool(name="const", bufs=1) as cpool,
            tc.tile_pool(name="xgh", bufs=14) as xhpool,
            tc.tile_pool(name="xgl", bufs=3) as xlpool,
            tc.tile_pool(name="agg", bufs=6, space="PSUM") as apsum,
            tc.tile_pool(name="ob", bufs=4) as opool,
        ):
            def slot_dma_h(s0, ns):
                ts = int(off[s0])
                nts = int(off[s0 + ns]) - ts
                t_ = xhpool.tile([BLK, nts * FH], ST_DT, tag="xh")
                # alternate HWDGE queues: overlaps the ~0.6us issue and
                # ~0.6us HWDGE fixed cost across two contexts
                eng = nc.sync if s0 % 2 == 0 else nc.scalar
                eng.dma_start(t_[:], xgh[:, ts * FH:(ts + nts) * FH])
                return t_

            def chunk_dma_l(s0):
                ts = int(off[s0])
                nts = int(off[s0 + CHUNK_SLOTS]) - ts
                t_ = xlpool.tile([BLK, nts * FL], FP8_DT, tag="xl")
                nc.scalar.dma_start(t_[:], xgl[:, ts * FL:(ts + nts) * FL])
                return t_

            # stream DMA units: one per slot
            units = [(s, 1) for s in range(SLOTS)]
            unit_of = {}
            unit_base = {}
            for ui, (s0, ns) in enumerate(units):
                for b in range(ns):
                    unit_of[s0 + b] = ui
                    unit_base[s0 + b] = s0
            unit_tiles = {0: slot_dma_h(*units[0])}
            chunk_l = {0: chunk_dma_l(0)} if FL else {}
            identb_sb = cpool.tile([BLK, BLK], ST_DT)
            nc.sync.dma_start(identb_sb[:], identb[:])
            if FL:
                identf_sb = cpool.tile([BLK, BLK], FP8_DT)
                nc.sync.dma_start(identf_sb[:], identf[:])

            def noload(mm):
                mm.ins.ldweights = False
                return mm

            def emit_out(s0, ps):
                ob = opool.tile([BLK, CHUNK_SLOTS, D_OUT], ST_DT, tag="ob")
                nc.vector.tensor_copy(out=ob[:], in_=ps[:])
                nc.scalar.dma_start(out[:, s0:s0 + CHUNK_SLOTS, :], ob[:])

            prev = None
            for ci in range(N_CHUNKS):
                s0 = ci * CHUNK_SLOTS
                for b in range(CHUNK_SLOTS):
                    ui = unit_of[s0 + b]
                    if ui not in unit_tiles:
                        unit_tiles[ui] = slot_dma_h(*units[ui])
                if FL and ci + 1 < N_CHUNKS and (s0 + CHUNK_SLOTS) not in chunk_l:
                    chunk_l[s0 + CHUNK_SLOTS] = chunk_dma_l(s0 + CHUNK_SLOTS)
                ps = apsum.tile([BLK, CHUNK_SLOTS, D_OUT], mybir.dt.float32,
                                tag="ps")
                if prev is not None:
                    # chunk ci-1's copy/store: deps a chunk old, stall-free
                    emit_out(*prev)
                # bf16 pass: one stationary load for the whole chunk
                nc.tensor.ldweights(identb_sb[:])
                for b in range(CHUNK_SLOTS):
                    s = s0 + b
                    xh_t = unit_tiles[unit_of[s]]
                    tb = int(off[s]) - int(off[unit_base[s]])
                    for t in range(T_list[s]):
                        ti = tb + t
                        noload(nc.tensor.matmul(
                            out=ps[:, b, 0:FH], lhsT=identb_sb[:],
                            rhs=xh_t[:, ti * FH:(ti + 1) * FH],
                            start=(t == 0), stop=(t == T_list[s] - 1)))
                if FL:
                    # fp8 pass
                    xl_t = chunk_l.pop(s0)
                    nc.tensor.ldweights(identf_sb[:])
                    for b in range(CHUNK_SLOTS):
                        s = s0 + b
                        toff = int(off[s]) - int(off[s0])
                        for t in range(T_list[s]):
                            ti = toff + t
                            noload(nc.tensor.matmul(
                                out=ps[:, b, FH:D_OUT], lhsT=identf_sb[:],
                                rhs=xl_t[:, ti * FL:(ti + 1) * FL],
                                start=(t == 0), stop=(t == T_list[s] - 1)))
                prev = (s0, ps)
            emit_out(*prev)
    nc.compile()
    return nc


def prepare(row_index, column_index):
    """Host-side index-only planning: degree sort, block deal, slotting."""
    row = np.ascontiguousarray(row_index).astype(np.int64)
    col = np.ascontiguousarray(column_index).astype(np.int64)
    deg = np.bincount(row, minlength=N_NODES)
    order = np.argsort(-deg, kind="stable")          # rank -> node
    rank = np.empty(N_NODES, np.int64)
    rank[order] = np.arange(N_NODES)
    ds = deg[order]
    # block j's max degree is its first member (descending sort)
    T_blk = np.maximum(ds[::BLK], 1)                 # [NBLK]
    # slot s on every core holds one of blocks 8s..8s+7; block 8s is the
    # largest, so T_blk[8s] covers all cores.  The xg layout and T_list
    # follow P_ORDER (processing order: smallest slot first).
    T_slot = T_blk[::N_CORES].astype(np.int64)       # [SLOTS], by slot id
    pinv = np.empty(SLOTS, np.int64)
    pinv[np.asarray(P_ORDER)] = np.arange(SLOTS)     # slot -> processed pos
    T_list = T_slot[np.asarray(P_ORDER)]             # by processed pos
    off = np.concatenate([[0], np.cumsum(T_list)]).astype(np.int64)
    NT = int(off[-1])

    r = rank[row]
    j = r // BLK                                     # dest block
    lane = r % BLK
    core = j % N_CORES
    pos = pinv[j // N_CORES]                         # processed position
    starts = np.concatenate([[0], np.cumsum(deg)]).astype(np.int64)
    occ = np.arange(N_EDGES, dtype=np.int64) - starts[row]
    tilei = off[pos] + occ                           # occ < T_list[pos]
    gidx = np.full((N_CORES, BLK, NT), -1, np.int64)
    gidx[core, lane, tilei] = col
    return {"order": order, "T_list": T_list, "gidx": gidx, "NT": NT}


def inputs_a(X, weights):
    X_bf = np.ascontiguousarray(X).astype(NP_ST)
    w_bf = np.ascontiguousarray(weights).astype(NP_ST)
    return [{"xt": np.ascontiguousarray(X_bf[k * NPC:(k + 1) * NPC].T),
             "w": w_bf} for k in range(N_CORES)]


def inputs_b(xp_any, P):
    """Gather X'[col] into the slotted lane-major streams per core (bf16
    leading FH features; fp8 trailing FL features)."""
    xp_bf = np.ascontiguousarray(xp_any).astype(NP_ST)
    xp_hi = np.ascontiguousarray(xp_bf[:, :FH])
    if FL:
        xp_lo = np.ascontiguousarray(xp_bf[:, FH:]).astype(NP_FP8)
    identb = np.eye(BLK, dtype=np.float32).astype(NP_ST)
    identf = np.eye(BLK, dtype=np.float32).astype(NP_FP8)
    NT = P["NT"]
    maps = []
    for k in range(N_CORES):
        g = P["gidx"][k].ravel()                     # [BLK*NT]
        gc = np.maximum(g, 0)
        bad = g < 0
        hi = xp_hi[gc]                               # [BLK*NT, FH]
        hi[bad] = 0
        m = {"xgh": np.ascontiguousarray(hi.reshape(BLK, NT * FH)),
             "identb": identb}
        if FL:
            lo = xp_lo[gc]
            lo[bad] = 0
            m["xgl"] = np.ascontiguousarray(lo.reshape(BLK, NT * FL))
            m["identf"] = identf
        maps.append(m)
    return maps


def unshard(P, outs):
    """outs[k]: device out [BLK, SLOTS, D_OUT] -> full [N_NODES, D_OUT]."""
    order = P["order"]
    res = np.zeros((N_NODES, D_OUT), np.float32)
    lanes = np.arange(BLK)[:, None]
    porder = np.asarray(P_ORDER)
    for k in range(N_CORES):
        ob = np.asarray(outs[k], dtype=np.float32)
        jj = porder[None, :] * N_CORES + k               # block of position p
        rk = jj * BLK + lanes                            # [BLK, SLOTS] ranks
        valid = rk < N_NODES
        res[order[rk[valid]]] = ob[valid]
    return res


def kernel(X, weights, row_index, column_index):
    global LAST_RESULTS
    P = prepare(row_index, column_index)
    nc_a = build_program_a()
    res_a = run_bass_kernel_spmd(nc_a, inputs_a(X, weights),
                                 list(range(N_CORES)), trace=TRACE)
    # device xp is [p, chunk, f], node n = chunk*128 + p
    xp = np.concatenate(
        [np.asarray(res_a.results[k]["xp"]).transpose(1, 0, 2)
         .reshape(-1, D_OUT)[:NPC] for k in range(N_CORES)],
        axis=0)                                          # [N_NODES, D_OUT]
    in_b = inputs_b(xp, P)
    nc_b = build_program_b(P["T_list"])
    res_b = run_bass_kernel_spmd(nc_b, in_b,
                                 list(range(N_CORES)), trace=TRACE)
    LAST_RESULTS = [res_a, res_b]
    return unshard(P, [res_b.results[k]["out"] for k in range(N_CORES)])


# revision 39
# speedup vs baseline: 1.0529x; 1.0196x over previous
"""GCNConv (out = segsum((X@W)[col], row)) on 8 TRN2 NeuronCores — v3.

v2 aggregated in D_in space: it streamed host-gathered neighbor rows at
128 feats/edge (bf16, ~29MB/core) and was DMA-bound at ~104us
(~330GB/s/core HBM), with 61us of DVE mask generation and 40us of ACT
copy overhead hidden under the stream.

v3 transforms FIRST so the gathered stream carries D_out=64 feats/edge
— half the bytes — and restructures so no masks are needed at all:

  Launch A (~10us): X' = X @ W, node-sharded (core k owns rows
    [6250k, 6250(k+1))), W stationary, X'^T written straight from PSUM.
  Host (index ops only): destinations sorted by degree and dealt
    round-robin into 128-dest blocks, so same-rank blocks across cores
    have near-equal tile counts (shared SPMD program, ~2.5% padding);
    X'[col] is gathered per edge into a slotted lane-major stream where
    lane l of EVERY tile belongs to dest l of the block.
  Launch B (~45us): stream Xg' [128, NT*64] bf16; the segment sum for a
    block is plain PSUM accumulation of its tiles under an IDENTITY
    stationary (one [128,128] lhsT reused by all 802 matmuls): no
    per-tile DVE masks, no rr stream, one DVE copy + one out-DMA per
    7-block chunk.

Precision: bf16 stream/weights, f32 PSUM accumulation, bf16 out (host
casts to f32): rel err ~2.5e-3 vs the 2e-2 gate.
"""

import numpy as np
import ml_dtypes

import concourse.mybir as mybir
import concourse.tile as tile
from concourse import bacc
from concourse.bass_utils import run_bass_kernel_spmd

# ---- problem constants (must match the harness inputs) ----
N_NODES = 50000
N_EDGES = 800000
D_IN = 128
D_OUT = 64
N_CORES = 8

NPC = N_NODES // N_CORES                    # 6250: nodes/core in launch A
BLK = 128                                   # dests per block in launch B
NBLK = (N_NODES + BLK - 1) // BLK           # 391 dest blocks
SLOTS = (NBLK + N_CORES - 1) // N_CORES     # 49 block slots per core
CHUNK_SLOTS = 7                             # blocks per chunk (psum 1792B)
N_CHUNKS = SLOTS // CHUNK_SLOTS             # 7
# slot processing order: the smallest slot first (fast PE start), then
# descending sizes; out[:, p, :] holds slot P_ORDER[p] (host relabels)
P_ORDER = [SLOTS - 1] + list(range(SLOTS - 1))
A_N = 512                                   # launch-A moving width
A_NCH = (NPC + A_N - 1) // A_N              # 13

ST_DT = mybir.dt.bfloat16
NP_ST = ml_dtypes.bfloat16

# test.py can flip this to get a profiled run; results land in LAST_RESULTS.
TRACE = False
LAST_RESULTS = None                         # [res_a, res_b]

# feature split of the Xg' stream: FH leading features in bf16, FL=64-FH
# trailing features in fp8e4m3 (exact 0/1 identity, quantization error only
# on the fp8 block: rel err ~2.65e-2 * sqrt(FL/64) end-to-end).  (64, 0)
# disables fp8.  Requires non-self-loading matmuls (the per-chunk identity
# loads): stationary dtype alternates per pass, so per-matmul reloads
# would make the PE LDW-bound.
FH, FL = 64, 0


def build_program_a():
    """X' = X @ W for this core's 6250-node slice, node-major output.

    Node-major (out partition = node) keeps all 128 DVE/ACT lanes busy in
    the psum->sbuf copies (2x the elems/cycle of the 64-partition
    W-stationary orientation) and the host gather reads rows directly.
    """
    nc = bacc.Bacc("TRN2", target_bir_lowering=False, debug=False,
                   num_devices=N_CORES)
    xt = nc.dram_tensor("xt", [D_IN, NPC], ST_DT, kind="ExternalInput").ap()
    w = nc.dram_tensor("w", [D_IN, D_OUT], ST_DT, kind="ExternalInput").ap()
    NCH = (NPC + BLK - 1) // BLK               # 49 chunks of 128 nodes
    GCH = 8                                    # chunks per group (1 bank)
    # out stays in sbuf layout [p, chunk, f] (node n = chunk*128 + p, host
    # reshapes): per-partition contiguous runs ~1KB, no <512B DMA penalty
    xp = nc.dram_tensor("xp", [BLK, NCH, D_OUT], ST_DT,
                        kind="ExternalOutput").ap()
    with tile.TileContext(nc) as tc:
        with (
            tc.tile_pool(name="const", bufs=1) as cpool,
            tc.tile_pool(name="xt", bufs=1) as xpool,
            tc.tile_pool(name="ps", bufs=6, space="PSUM") as psum,
            tc.tile_pool(name="xo", bufs=4) as opool,
        ):
            w_sb = cpool.tile([D_IN, D_OUT], ST_DT)
            nc.sync.dma_start(w_sb[:], w[:])
            xt_sb = xpool.tile([D_IN, NPC], ST_DT)
            # alternate input DMAs across the two HWDGE queues: the ~0.6us
            # per-DMA sequencer issue time was serializing the input stream
            for i, n0 in enumerate(range(0, NPC, GCH * BLK)):
                ng = min(GCH * BLK, NPC - n0)
                eng = nc.sync if i % 2 == 0 else nc.scalar
                eng.dma_start(xt_sb[:, n0:n0 + ng], xt[:, n0:n0 + ng])
            for g in range(0, NCH, GCH):
                nch = min(GCH, NCH - g)
                ps = psum.tile([BLK, GCH, D_OUT], mybir.dt.float32, tag="ps")
                for c in range(nch):
                    n0 = (g + c) * BLK
                    nn = min(BLK, NPC - n0)
                    nc.tensor.matmul(out=ps[:nn, c, :], lhsT=xt_sb[:, n0:n0 + nn],
                                     rhs=w_sb[:], start=True, stop=True)
                xo = opool.tile([BLK, GCH, D_OUT], ST_DT, tag="xo")
                np_ = min(BLK, NPC - (g + nch - 1) * BLK)  # last-chunk rows
                np_ = BLK if nch > 1 else np_
                if (g // GCH) % 2 == 0:
                    nc.vector.tensor_copy(out=xo[:np_, :nch, :],
                                          in_=ps[:np_, :nch, :])
                else:
                    nc.scalar.copy(xo[:np_, :nch, :], ps[:np_, :nch, :])
                eng = nc.sync if (g // GCH) % 2 == 0 else nc.scalar
                eng.dma_start(xp[:np_, g:g + nch, :], xo[:np_, :nch, :])
    nc.compile()
    return nc


FP8_DT = mybir.dt.float8e4
NP_FP8 = ml_dtypes.float8_e4m3fn


def build_program_b(T_list):
    """Segment-sum of the slotted Xg' stream: identity-stationary matmuls.

    T_list[s] = tiles for block slot s (uniform across cores; processing
    order).  Per chunk: a bf16 pass (FH leading feats, per-slot HWDGE
    DMAs on SP) and an fp8 pass (FL trailing feats, per-chunk DMAs on the
    DVE queue), each under ONE explicit identity ldweights; the matmuls
    are non-self-loading.
    """
    T_list = [int(t) for t in T_list]
    off = np.concatenate([[0], np.cumsum(T_list)]).astype(int)
    nc = bacc.Bacc("TRN2", target_bir_lowering=False, debug=False,
                   num_devices=N_CORES)
    NT = int(off[-1])
    xgh = nc.dram_tensor("xgh", [BLK, NT * FH], ST_DT,
                         kind="ExternalInput").ap()
    identb = nc.dram_tensor("identb", [BLK, BLK], ST_DT,
                            kind="ExternalInput").ap()
    if FL:
        xgl = nc.dram_tensor("xgl", [BLK, NT * FL], FP8_DT,
                             kind="ExternalInput").ap()
        identf = nc.dram_tensor("identf", [BLK, BLK], FP8_DT,
                                kind="ExternalInput").ap()
    # out[lane, p, f']; host maps (lane, p) -> node via P_ORDER/degree sort
    out = nc.dram_tensor("out", [BLK, SLOTS, D_OUT], ST_DT,
                         kind="ExternalOutput").ap()

    with tile.TileContext(nc) as tc:
        with (
            tc.tile_pool(name="const", bufs=1) as cpool,
            tc.tile_pool(name="xgh", bufs=20) as xhpool,
            tc.tile_pool(name="xgl", bufs=3) as xlpool,
            tc.tile_pool(name="agg", bufs=6, space="PSUM") as apsum,
            tc.tile_pool(name="ob", bufs=4) as opool,
        ):
            def slot_dma_h(s0, ns):
                ts = int(off[s0])
                nts = int(off[s0 + ns]) - ts
                t_ = xhpool.tile([BLK, nts * FH], ST_DT, tag="xh")
                # alternate HWDGE queues: overlaps the ~0.6us issue and
                # ~0.6us HWDGE fixed cost across two contexts
                eng = nc.sync if s0 % 2 == 0 else nc.scalar
                eng.dma_start(t_[:], xgh[:, ts * FH:(ts + nts) * FH])
                return t_

            def chunk_dma_l(s0):
                ts = int(off[s0])
                nts = int(off[s0 + CHUNK_SLOTS]) - ts
                t_ = xlpool.tile([BLK, nts * FL], FP8_DT, tag="xl")
                nc.scalar.dma_start(t_[:], xgl[:, ts * FL:(ts + nts) * FL])
                return t_

            # stream DMA units: one per slot
            units = [(s, 1) for s in range(SLOTS)]
            unit_of = {}
            unit_base = {}
            for ui, (s0, ns) in enumerate(units):
                for b in range(ns):
                    unit_of[s0 + b] = ui
                    unit_base[s0 + b] = s0
            unit_tiles = {0: slot_dma_h(*units[0])}
            chunk_l = {0: chunk_dma_l(0)} if FL else {}
            identb_sb = cpool.tile([BLK, BLK], ST_DT)
            nc.sync.dma_start(identb_sb[:], identb[:])
            if FL:
                identf_sb = cpool.tile([BLK, BLK], FP8_DT)
                nc.sync.dma_start(identf_sb[:], identf[:])

            def noload(mm):
                mm.ins.ldweights = False
                return mm

            def emit_out(s0, ps):
                ob = opool.tile([BLK, CHUNK_SLOTS, D_OUT], ST_DT, tag="ob")
                nc.vector.tensor_copy(out=ob[:], in_=ps[:])
                nc.scalar.dma_start(out[:, s0:s0 + CHUNK_SLOTS, :], ob[:])

            prev = None
            for ci in range(N_CHUNKS):
                s0 = ci * CHUNK_SLOTS
                for b in range(CHUNK_SLOTS):
                    ui = unit_of[s0 + b]
                    if ui not in unit_tiles:
                        unit_tiles[ui] = slot_dma_h(*units[ui])
                if FL and ci + 1 < N_CHUNKS and (s0 + CHUNK_SLOTS) not in chunk_l:
                    chunk_l[s0 + CHUNK_SLOTS] = chunk_dma_l(s0 + CHUNK_SLOTS)
                ps = apsum.tile([BLK, CHUNK_SLOTS, D_OUT], mybir.dt.float32,
                                tag="ps")
                if prev is not None:
                    # chunk ci-1's copy/store: deps a chunk old, stall-free
                    emit_out(*prev)
                # bf16 pass: one stationary load for the whole chunk
                nc.tensor.ldweights(identb_sb[:])
                for b in range(CHUNK_SLOTS):
                    s = s0 + b
                    xh_t = unit_tiles[unit_of[s]]
                    tb = int(off[s]) - int(off[unit_base[s]])
                    for t in range(T_list[s]):
                        ti = tb + t
                        noload(nc.tensor.matmul(
                            out=ps[:, b, 0:FH], lhsT=identb_sb[:],
                            rhs=xh_t[:, ti * FH:(ti + 1) * FH],
                            start=(t == 0), stop=(t == T_list[s] - 1)))
                if FL:
                    # fp8 pass
                    xl_t = chunk_l.pop(s0)
                    nc.tensor.ldweights(identf_sb[:])
                    for b in range(CHUNK_SLOTS):
                        s = s0 + b
                        toff = int(off[s]) - int(off[s0])
                        for t in range(T_list[s]):
                            ti = toff + t
                            noload(nc.tensor.matmul(
                                out=ps[:, b, FH:D_OUT], lhsT=identf_sb[:],
                                rhs=xl_t[:, ti * FL:(ti + 1) * FL],
                                start=(t == 0), stop=(t == T_list[s] - 1)))
                prev = (s0, ps)
            emit_out(*prev)
    nc.compile()
    return nc


def prepare(row_index, column_index):
    """Host-side index-only planning: degree sort, block deal, slotting."""
    row = np.ascontiguousarray(row_index).astype(np.int64)
    col = np.ascontiguousarray(column_index).astype(np.int64)
    deg = np.bincount(row, minlength=N_NODES)
    order = np.argsort(-deg, kind="stable")          # rank -> node
    rank = np.empty(N_NODES, np.int64)
    rank[order] = np.arange(N_NODES)
    ds = deg[order]
    # block j's max degree is its first member (descending sort)
    T_blk = np.maximum(ds[::BLK], 1)                 # [NBLK]
    # slot s on every core holds one of blocks 8s..8s+7; block 8s is the
    # largest, so T_blk[8s] covers all cores.  The xg layout and T_list
    # follow P_ORDER (processing order: smallest slot first).
    T_slot = T_blk[::N_CORES].astype(np.int64)       # [SLOTS], by slot id
    pinv = np.empty(SLOTS, np.int64)
    pinv[np.asarray(P_ORDER)] = np.arange(SLOTS)     # slot -> processed pos
    T_list = T_slot[np.asarray(P_ORDER)]             # by processed pos
    off = np.concatenate([[0], np.cumsum(T_list)]).astype(np.int64)
    NT = int(off[-1])

    r = rank[row]
    j = r // BLK                                     # dest block
    lane = r % BLK
    core = j % N_CORES
    pos = pinv[j // N_CORES]                         # processed position
    starts = np.concatenate([[0], np.cumsum(deg)]).astype(np.int64)
    occ = np.arange(N_EDGES, dtype=np.int64) - starts[row]
    tilei = off[pos] + occ                           # occ < T_list[pos]
    gidx = np.full((N_CORES, BLK, NT), -1, np.int64)
    gidx[core, lane, tilei] = col
    return {"order": order, "T_list": T_list, "gidx": gidx, "NT": NT}


def inputs_a(X, weights):
    X_bf = np.ascontiguousarray(X).astype(NP_ST)
    w_bf = np.ascontiguousarray(weights).astype(NP_ST)
    return [{"xt": np.ascontiguousarray(X_bf[k * NPC:(k + 1) * NPC].T),
             "w": w_bf} for k in range(N_CORES)]


def inputs_b(xp_any, P):
    """Gather X'[col] into the slotted lane-major streams per core (bf16
    leading FH features; fp8 trailing FL features)."""
    xp_bf = np.ascontiguousarray(xp_any).astype(NP_ST)
    xp_hi = np.ascontiguousarray(xp_bf[:, :FH])
    if FL:
        xp_lo = np.ascontiguousarray(xp_bf[:, FH:]).astype(NP_FP8)
    identb = np.eye(BLK, dtype=np.float32).astype(NP_ST)
    identf = np.eye(BLK, dtype=np.float32).astype(NP_FP8)
    NT = P["NT"]
    maps = []
    for k in range(N_CORES):
        g = P["gidx"][k].ravel()                     # [BLK*NT]
        gc = np.maximum(g, 0)
        bad = g < 0
        hi = xp_hi[gc]                               # [BLK*NT, FH]
        hi[bad] = 0
        m = {"xgh": np.ascontiguousarray(hi.reshape(BLK, NT * FH)),
             "identb": identb}
        if FL:
            lo = xp_lo[gc]
            lo[bad] = 0
            m["xgl"] = np.ascontiguousarray(lo.reshape(BLK, NT * FL))
            m["identf"] = identf
        maps.append(m)
    return maps


def unshard(P, outs):
    """outs[k]: device out [BLK, SLOTS, D_OUT] -> full [N_NODES, D_OUT]."""
    order = P["order"]
    res = np.zeros((N_NODES, D_OUT), np.float32)
    lanes = np.arange(BLK)[:, None]
    porder = np.asarray(P_ORDER)
    for k in range(N_CORES):
        ob = np.asarray(outs[k], dtype=np.float32)
        jj = porder[None, :] * N_CORES + k               # block of position p
        rk = jj * BLK + lanes                            # [BLK, SLOTS] ranks
        valid = rk < N_NODES
        res[order[rk[valid]]] = ob[valid]
    return res


def kernel(X, weights, row_index, column_index):
    global LAST_RESULTS
    P = prepare(row_index, column_index)
    nc_a = build_program_a()
    res_a = run_bass_kernel_spmd(nc_a, inputs_a(X, weights),
                                 list(range(N_CORES)), trace=TRACE)
    # device xp is [p, chunk, f], node n = chunk*128 + p
    xp = np.concatenate(
        [np.asarray(res_a.results[k]["xp"]).transpose(1, 0, 2)
         .reshape(-1, D_OUT)[:NPC] for k in range(N_CORES)],
        axis=0)                                          # [N_NODES, D_OUT]
    in_b = inputs_b(xp, P)
    nc_b = build_program_b(P["T_list"])
    res_b = run_bass_kernel_spmd(nc_b, in_b,
                                 list(range(N_CORES)), trace=TRACE)
    LAST_RESULTS = [res_a, res_b]
    return unshard(P, [res_b.results[k]["out"] for k in range(N_CORES)])
